# revision 9
# baseline (speedup 1.0000x reference)
import sys

sys.path.insert(0, "/opt/trn_rl_repo")
import numpy as np

N1, N2, D = 8192, 8192, 256
NCORES = 8
QPC = N1 // NCORES  # queries per core (1024)
RT = QPC // 128  # row tiles per core (8)
GW = 2048  # group width (4 psum banks)
NEG = -1.0e30


def _build_nc():
    import concourse.bass as bass
    import concourse.tile as tile
    from concourse import mybir

    f32, f32r = mybir.dt.float32, mybir.dt.float32r
    nc = bass.Bass()
    dbx = nc.dram_tensor("dbx", [128, 2, 2 * N1], f32r, kind="ExternalInput")
    nrmA = nc.dram_tensor("nrmA", [2, N1 + 128], f32r, kind="ExternalInput")
    nrmB = nc.dram_tensor("nrmB", [2, N1], f32r, kind="ExternalInput")
    dmask = nc.dram_tensor("dmask", [128, 4 * 512], f32, kind="ExternalInput")
    o = nc.dram_tensor("o", [128, RT, 8], f32, kind="ExternalOutput")

    with tile.TileContext(nc) as tc:
        with (
            tc.tile_pool(name="sb", bufs=1) as sb,
            tc.tile_pool(name="pp", bufs=1) as pp,
            tc.tile_pool(name="ps", bufs=2, space="PSUM") as ps,
        ):
            CW = 2 * GW  # chunk width (4096 cols)
            chunks = {}
            for side in range(2):
                for h in range(2):
                    t = sb.tile([128, 2, CW], f32r, name=f"db{side}{h}", tag=f"db{side}{h}")
                    off = side * N1 + h * CW
                    eng = [nc.sync, nc.scalar, nc.sync, nc.scalar][side * 2 + h]
                    eng.dma_start(out=t, in_=dbx[:, :, off : off + CW])
                    chunks[(side, h)] = t
            tnrA = sb.tile([2, N1 + 128], f32r, tag="nrA")
            tnrB = sb.tile([2, N1], f32r, tag="nrB")
            tmk = sb.tile([128, 4 * 512], f32, tag="mk")
            nc.sync.dma_start(out=tnrA, in_=nrmA[:])
            nc.sync.dma_start(out=tnrB, in_=nrmB[:])
            nc.sync.dma_start(out=tmk, in_=dmask[:])
            ones2 = tnrA[0:2, N1 : N1 + 128]
            # wait absorber: DVE observes the dmask DMA once, up front
            dum = sb.tile([128, 1], f32, tag="dum")
            nc.vector.tensor_copy(out=dum, in_=tmk[:, 0:1])
            parts = [pp.tile([128, 8], f32, name=f"part{m}", tag=f"part{m}") for m in range(RT)]
            tq = chunks[(0, 0)]
            for side in range(2):
                for h in range(2):
                    tch = chunks[(side, h)]
                    for m in range(RT):
                        lhs = [tq[:, k, m * 128 : (m + 1) * 128] for k in (0, 1)]
                        part = parts[m]
                        for g2 in range(2):
                            g = h * 2 + g2
                            col = g * GW
                            lcol = g2 * GW
                            pst = ps.tile([128, GW], f32, tag="pst")
                            for k in (0, 1):
                                for i in range(4):
                                    nc.tensor.matmul(
                                        out=pst[:, i * 512 : (i + 1) * 512],
                                        lhsT=lhs[k],
                                        rhs=tch[
                                            :, k, lcol + i * 512 : lcol + (i + 1) * 512
                                        ],
                                        start=(k == 0),
                                        stop=False,
                                    )
                            for i in range(4):
                                nc.tensor.matmul(
                                    out=pst[:, i * 512 : (i + 1) * 512],
                                    lhsT=ones2,
                                    rhs=(tnrA if side == 0 else tnrB)[
                                        0:2, col + i * 512 : col + (i + 1) * 512
                                    ],
                                    start=False,
                                    stop=True,
                                )
                            if side == 0 and g == 0:
                                i0, v = m // 4, m % 4
                                sl = pst[:, i0 * 512 : (i0 + 1) * 512]
                                nc.vector.tensor_add(
                                    out=sl, in0=sl, in1=tmk[:, v * 512 : (v + 1) * 512]
                                )
                            nc.vector.tensor_reduce(
                                out=part[:, side * 4 + g : side * 4 + g + 1],
                                in_=pst,
                                axis=mybir.AxisListType.X,
                                op=mybir.AluOpType.max,
                            )
            for m in range(RT):
                nc.sync.dma_start(out=o[:, m, :], in_=parts[m])

    from concourse.bass import _bass_rust

    _bass_rust.move_matmul_waits_to_ldweights(nc.m)
    _bass_rust.generate_event_semaphores(nc)
    return nc


def _tf32_hi(x):
    return (x.astype(np.float32).view(np.uint32) & 0xFFFFE000).view(np.float32)


def _prep_core(s1, s2T, sq2hi, sq2lo, c):
    s1p = np.roll(s1, -c * QPC, axis=0)
    dbx = np.empty((128, 2, 2 * N1), dtype=np.float32)
    s1pT = np.ascontiguousarray(s1p.T)
    for k in (0, 1):
        dbx[:, k, 0:N1] = s1pT[k * 128 : (k + 1) * 128]
        dbx[:, k, N1 : 2 * N1] = s2T[k * 128 : (k + 1) * 128]
    nA = (-0.5 * np.square(s1p.astype(np.float64)).sum(1)).astype(np.float32)
    hiA = _tf32_hi(nA)
    nrmA = np.ones((2, N1 + 128), dtype=np.float32)
    nrmA[0, :N1], nrmA[1, :N1] = hiA, nA - hiA
    return dbx, nrmA


def kernel(s1, s2, k):
    assert int(k) == 1
    from concourse.bass_utils import run_bass_kernel_spmd

    s1 = np.asarray(s1, dtype=np.float32)
    s2 = np.asarray(s2, dtype=np.float32)
    nB = (-0.5 * np.square(s2.astype(np.float64)).sum(1)).astype(np.float32)
    hiB = _tf32_hi(nB)
    nrmB = np.stack([hiB, nB - hiB])
    s2T = np.ascontiguousarray(s2.T)
    dmask = np.zeros((128, 4 * 512), dtype=np.float32)
    for v in range(4):
        for p in range(128):
            dmask[p, v * 512 + v * 128 + p] = NEG

    nc = _build_nc()
    in_maps = []
    for c in range(NCORES):
        dbx, nrmA = _prep_core(s1, s2T, hiB, nB - hiB, c)
        in_maps.append({"dbx": dbx, "nrmA": nrmA, "nrmB": nrmB, "dmask": dmask})
    import os
    res = run_bass_kernel_spmd(
        nc, in_maps, core_ids=list(range(NCORES)),
        trace=os.environ.get("KBENCH_TRACE") == "1",
    )
    kernel.last_results = res

    # host epilogue (float64): rho/nu from per-group maxes, then the estimator
    sq1 = np.square(s1.astype(np.float64)).sum(1)
    total = 0.0
    for c in range(NCORES):
        part = res.results[c]["o"].astype(np.float64)  # [128, RT, 8]
        maxA = part[:, :, 0:4].max(axis=2)  # [128, RT]
        maxB = part[:, :, 4:8].max(axis=2)
        idx = np.arange(RT)[None, :] * 128 + np.arange(128)[:, None]
        orig = (c * QPC + idx) % N1
        sqx = sq1[orig]
        rho_sq = sqx - 2.0 * maxA
        nu_sq = sqx - 2.0 * maxB
        rho_sq = np.maximum(rho_sq, 1e-20)
        nu_sq = np.maximum(nu_sq, 1e-20)
        total += 0.5 * (np.log(nu_sq) - np.log(rho_sq)).sum()
    base = np.log(N2 / (N1 - 1))
    return np.float32(base + (D / N1) * total)


# revision 10
# speedup vs baseline: 1.0707x; 1.0707x over previous
import sys

sys.path.insert(0, "/opt/trn_rl_repo")
import numpy as np

N1, N2, D = 8192, 8192, 256
NCORES = 8
QPC = N1 // NCORES  # queries per core (1024)
RT = QPC // 128  # row tiles per core (8)
GW = 2048  # group width (4 psum banks)
NEG = -1.0e30


def _build_nc():
    import concourse.bass as bass
    import concourse.tile as tile
    from concourse import mybir

    f32, f32r = mybir.dt.float32, mybir.dt.float32r
    nc = bass.Bass()
    dbx = nc.dram_tensor("dbx", [128, 2, 2 * N1], f32r, kind="ExternalInput")
    nrmA = nc.dram_tensor("nrmA", [2, N1 + 128], f32r, kind="ExternalInput")
    nrmB = nc.dram_tensor("nrmB", [2, N1], f32r, kind="ExternalInput")
    dmask = nc.dram_tensor("dmask", [128, 4 * 512], f32, kind="ExternalInput")
    o = nc.dram_tensor("o", [128, RT, 8], f32, kind="ExternalOutput")

    with tile.TileContext(nc) as tc:
        with (
            tc.tile_pool(name="sb", bufs=1) as sb,
            tc.tile_pool(name="pp", bufs=1) as pp,
            tc.tile_pool(name="ps", bufs=2, space="PSUM") as ps,
        ):
            CW = 2 * GW  # chunk width (4096 cols)
            # small inputs first: the first norm-MM/mask-TT consumers must not
            # queue behind the 16MB database load on the same DMA engines
            tnrA = sb.tile([2, N1 + 128], f32r, tag="nrA")
            tnrB = sb.tile([2, N1], f32r, tag="nrB")
            tmk = sb.tile([128, 4 * 512], f32, tag="mk")
            nc.sync.dma_start(out=tnrA, in_=nrmA[:])
            nc.sync.dma_start(out=tnrB, in_=nrmB[:])
            nc.scalar.dma_start(out=tmk, in_=dmask[:])
            chunks = {}
            for side in range(2):
                for h in range(2):
                    t = sb.tile([128, 2, CW], f32r, name=f"db{side}{h}", tag=f"db{side}{h}")
                    off = side * N1 + h * CW
                    eng = [nc.sync, nc.scalar, nc.sync, nc.scalar][side * 2 + h]
                    eng.dma_start(out=t, in_=dbx[:, :, off : off + CW])
                    chunks[(side, h)] = t
            ones2 = tnrA[0:2, N1 : N1 + 128]
            # wait absorber: DVE observes the dmask DMA once, up front
            dum = sb.tile([128, 1], f32, tag="dum")
            nc.vector.tensor_copy(out=dum, in_=tmk[:, 0:1])
            parts = [pp.tile([128, 8], f32, name=f"part{m}", tag=f"part{m}") for m in range(RT)]
            tq = chunks[(0, 0)]
            for side in range(2):
                for h in range(2):
                    tch = chunks[(side, h)]
                    for m in range(RT):
                        lhs = [tq[:, k, m * 128 : (m + 1) * 128] for k in (0, 1)]
                        part = parts[m]
                        for g2 in range(2):
                            g = h * 2 + g2
                            col = g * GW
                            lcol = g2 * GW
                            pst = ps.tile([128, GW], f32, tag="pst")
                            for k in (0, 1):
                                for i in range(4):
                                    nc.tensor.matmul(
                                        out=pst[:, i * 512 : (i + 1) * 512],
                                        lhsT=lhs[k],
                                        rhs=tch[
                                            :, k, lcol + i * 512 : lcol + (i + 1) * 512
                                        ],
                                        start=(k == 0),
                                        stop=False,
                                    )
                            for i in range(4):
                                nc.tensor.matmul(
                                    out=pst[:, i * 512 : (i + 1) * 512],
                                    lhsT=ones2,
                                    rhs=(tnrA if side == 0 else tnrB)[
                                        0:2, col + i * 512 : col + (i + 1) * 512
                                    ],
                                    start=False,
                                    stop=True,
                                )
                            if side == 0 and g == 0:
                                i0, v = m // 4, m % 4
                                sl = pst[:, i0 * 512 : (i0 + 1) * 512]
                                nc.vector.tensor_add(
                                    out=sl, in0=sl, in1=tmk[:, v * 512 : (v + 1) * 512]
                                )
                            nc.vector.tensor_reduce(
                                out=part[:, side * 4 + g : side * 4 + g + 1],
                                in_=pst,
                                axis=mybir.AxisListType.X,
                                op=mybir.AluOpType.max,
                            )
            for m in range(RT):
                nc.sync.dma_start(out=o[:, m, :], in_=parts[m])

    from concourse.bass import _bass_rust

    _bass_rust.move_matmul_waits_to_ldweights(nc.m)
    _bass_rust.generate_event_semaphores(nc)
    return nc


def _tf32_hi(x):
    return (x.astype(np.float32).view(np.uint32) & 0xFFFFE000).view(np.float32)


def _prep_core(s1, s2T, sq2hi, sq2lo, c):
    s1p = np.roll(s1, -c * QPC, axis=0)
    dbx = np.empty((128, 2, 2 * N1), dtype=np.float32)
    s1pT = np.ascontiguousarray(s1p.T)
    for k in (0, 1):
        dbx[:, k, 0:N1] = s1pT[k * 128 : (k + 1) * 128]
        dbx[:, k, N1 : 2 * N1] = s2T[k * 128 : (k + 1) * 128]
    nA = (-0.5 * np.square(s1p.astype(np.float64)).sum(1)).astype(np.float32)
    hiA = _tf32_hi(nA)
    nrmA = np.ones((2, N1 + 128), dtype=np.float32)
    nrmA[0, :N1], nrmA[1, :N1] = hiA, nA - hiA
    return dbx, nrmA


def kernel(s1, s2, k):
    assert int(k) == 1
    from concourse.bass_utils import run_bass_kernel_spmd

    s1 = np.asarray(s1, dtype=np.float32)
    s2 = np.asarray(s2, dtype=np.float32)
    nB = (-0.5 * np.square(s2.astype(np.float64)).sum(1)).astype(np.float32)
    hiB = _tf32_hi(nB)
    nrmB = np.stack([hiB, nB - hiB])
    s2T = np.ascontiguousarray(s2.T)
    dmask = np.zeros((128, 4 * 512), dtype=np.float32)
    for v in range(4):
        for p in range(128):
            dmask[p, v * 512 + v * 128 + p] = NEG

    nc = _build_nc()
    in_maps = []
    for c in range(NCORES):
        dbx, nrmA = _prep_core(s1, s2T, hiB, nB - hiB, c)
        in_maps.append({"dbx": dbx, "nrmA": nrmA, "nrmB": nrmB, "dmask": dmask})
    import os
    res = run_bass_kernel_spmd(
        nc, in_maps, core_ids=list(range(NCORES)),
        trace=os.environ.get("KBENCH_TRACE") == "1",
    )
    kernel.last_results = res

    # host epilogue (float64): rho/nu from per-group maxes, then the estimator
    sq1 = np.square(s1.astype(np.float64)).sum(1)
    total = 0.0
    for c in range(NCORES):
        part = res.results[c]["o"].astype(np.float64)  # [128, RT, 8]
        maxA = part[:, :, 0:4].max(axis=2)  # [128, RT]
        maxB = part[:, :, 4:8].max(axis=2)
        idx = np.arange(RT)[None, :] * 128 + np.arange(128)[:, None]
        orig = (c * QPC + idx) % N1
        sqx = sq1[orig]
        rho_sq = sqx - 2.0 * maxA
        nu_sq = sqx - 2.0 * maxB
        rho_sq = np.maximum(rho_sq, 1e-20)
        nu_sq = np.maximum(nu_sq, 1e-20)
        total += 0.5 * (np.log(nu_sq) - np.log(rho_sq)).sum()
    base = np.log(N2 / (N1 - 1))
    return np.float32(base + (D / N1) * total)


# revision 11
# speedup vs baseline: 1.1261x; 1.0518x over previous
import sys

sys.path.insert(0, "/opt/trn_rl_repo")
import numpy as np

N1, N2, D = 8192, 8192, 256
NCORES = 8
QPC = N1 // NCORES  # queries per core (1024)
RT = QPC // 128  # row tiles per core (8)
GW = 2048  # group width (4 psum banks)
NEG = -1.0e30


def _build_nc():
    import concourse.bass as bass
    import concourse.tile as tile
    from concourse import mybir

    f32, f32r = mybir.dt.float32, mybir.dt.float32r
    nc = bass.Bass()
    dbx = nc.dram_tensor("dbx", [128, 2, 2 * N1], f32r, kind="ExternalInput")
    nrmA = nc.dram_tensor("nrmA", [2, N1 + 128], f32r, kind="ExternalInput")
    nrmB = nc.dram_tensor("nrmB", [2, N1], f32r, kind="ExternalInput")
    dmask = nc.dram_tensor("dmask", [128, 4 * 512], f32, kind="ExternalInput")
    o = nc.dram_tensor("o", [128, RT, 8], f32, kind="ExternalOutput")

    with tile.TileContext(nc) as tc:
        with (
            tc.tile_pool(name="sb", bufs=1) as sb,
            tc.tile_pool(name="pp", bufs=1) as pp,
            tc.tile_pool(name="ps", bufs=2, space="PSUM") as ps,
        ):
            CW = 2 * GW  # chunk width (4096 cols)
            # small inputs first: the first norm-MM/mask-TT consumers must not
            # queue behind the 16MB database load on the same DMA engines
            tnrA = sb.tile([2, N1 + 128], f32r, tag="nrA")
            tnrB = sb.tile([2, N1], f32r, tag="nrB")
            tmk = sb.tile([128, 4 * 512], f32, tag="mk")
            nc.sync.dma_start(out=tnrA, in_=nrmA[:])
            nc.sync.dma_start(out=tnrB, in_=nrmB[:])
            nc.scalar.dma_start(out=tmk, in_=dmask[:])
            # first chunk split in half across both HWDGE engines so the
            # opening matmul group's data lands ASAP
            t000 = sb.tile([128, 2, GW], f32r, name="db000", tag="db000")
            nc.sync.dma_start(out=t000, in_=dbx[:, :, 0:GW])
            t00b = sb.tile([128, 2, GW], f32r, name="db00b", tag="db00b")
            nc.scalar.dma_start(out=t00b, in_=dbx[:, :, GW:CW])
            chunks = {(0, 0): (t000, t00b)}
            for side, h in [(0, 1), (1, 0), (1, 1)]:
                t = sb.tile([128, 2, CW], f32r, name=f"db{side}{h}", tag=f"db{side}{h}")
                off = side * N1 + h * CW
                eng = [None, nc.sync, nc.scalar, nc.sync][side * 2 + h]
                eng.dma_start(out=t, in_=dbx[:, :, off : off + CW])
                chunks[(side, h)] = t
            ones2 = tnrA[0:2, N1 : N1 + 128]
            # wait absorber: DVE observes the dmask DMA once, up front
            dum = sb.tile([128, 1], f32, tag="dum")
            nc.vector.tensor_copy(out=dum, in_=tmk[:, 0:1])
            parts = [pp.tile([128, 8], f32, name=f"part{m}", tag=f"part{m}") for m in range(RT)]
            tq = t000
            for side in range(2):
                for h in range(2):
                    tch = chunks[(side, h)]
                    for m in range(RT):
                        lhs = [tq[:, k, m * 128 : (m + 1) * 128] for k in (0, 1)]
                        part = parts[m]
                        for g2 in range(2):
                            g = h * 2 + g2
                            col = g * GW
                            if isinstance(tch, tuple):
                                tsrc, lcol = tch[g2], 0
                            else:
                                tsrc, lcol = tch, g2 * GW
                            pst = ps.tile([128, GW], f32, tag="pst")
                            for k in (0, 1):
                                for i in range(4):
                                    nc.tensor.matmul(
                                        out=pst[:, i * 512 : (i + 1) * 512],
                                        lhsT=lhs[k],
                                        rhs=tsrc[
                                            :, k, lcol + i * 512 : lcol + (i + 1) * 512
                                        ],
                                        start=(k == 0),
                                        stop=False,
                                    )
                            for i in range(4):
                                nc.tensor.matmul(
                                    out=pst[:, i * 512 : (i + 1) * 512],
                                    lhsT=ones2,
                                    rhs=(tnrA if side == 0 else tnrB)[
                                        0:2, col + i * 512 : col + (i + 1) * 512
                                    ],
                                    start=False,
                                    stop=True,
                                )
                            if side == 0 and g == 0:
                                i0, v = m // 4, m % 4
                                sl = pst[:, i0 * 512 : (i0 + 1) * 512]
                                nc.vector.tensor_add(
                                    out=sl, in0=sl, in1=tmk[:, v * 512 : (v + 1) * 512]
                                )
                            nc.vector.tensor_reduce(
                                out=part[:, side * 4 + g : side * 4 + g + 1],
                                in_=pst,
                                axis=mybir.AxisListType.X,
                                op=mybir.AluOpType.max,
                            )
            for m in range(RT):
                nc.sync.dma_start(out=o[:, m, :], in_=parts[m])

    from concourse.bass import _bass_rust

    _bass_rust.move_matmul_waits_to_ldweights(nc.m)
    _bass_rust.generate_event_semaphores(nc)
    return nc


def _tf32_hi(x):
    return (x.astype(np.float32).view(np.uint32) & 0xFFFFE000).view(np.float32)


def _prep_core(s1, s2T, sq2hi, sq2lo, c):
    s1p = np.roll(s1, -c * QPC, axis=0)
    dbx = np.empty((128, 2, 2 * N1), dtype=np.float32)
    s1pT = np.ascontiguousarray(s1p.T)
    for k in (0, 1):
        dbx[:, k, 0:N1] = s1pT[k * 128 : (k + 1) * 128]
        dbx[:, k, N1 : 2 * N1] = s2T[k * 128 : (k + 1) * 128]
    nA = (-0.5 * np.square(s1p.astype(np.float64)).sum(1)).astype(np.float32)
    hiA = _tf32_hi(nA)
    nrmA = np.ones((2, N1 + 128), dtype=np.float32)
    nrmA[0, :N1], nrmA[1, :N1] = hiA, nA - hiA
    return dbx, nrmA


def kernel(s1, s2, k):
    assert int(k) == 1
    from concourse.bass_utils import run_bass_kernel_spmd

    s1 = np.asarray(s1, dtype=np.float32)
    s2 = np.asarray(s2, dtype=np.float32)
    nB = (-0.5 * np.square(s2.astype(np.float64)).sum(1)).astype(np.float32)
    hiB = _tf32_hi(nB)
    nrmB = np.stack([hiB, nB - hiB])
    s2T = np.ascontiguousarray(s2.T)
    dmask = np.zeros((128, 4 * 512), dtype=np.float32)
    for v in range(4):
        for p in range(128):
            dmask[p, v * 512 + v * 128 + p] = NEG

    nc = _build_nc()
    in_maps = []
    for c in range(NCORES):
        dbx, nrmA = _prep_core(s1, s2T, hiB, nB - hiB, c)
        in_maps.append({"dbx": dbx, "nrmA": nrmA, "nrmB": nrmB, "dmask": dmask})
    import os
    res = run_bass_kernel_spmd(
        nc, in_maps, core_ids=list(range(NCORES)),
        trace=os.environ.get("KBENCH_TRACE") == "1",
    )
    kernel.last_results = res

    # host epilogue (float64): rho/nu from per-group maxes, then the estimator
    sq1 = np.square(s1.astype(np.float64)).sum(1)
    total = 0.0
    for c in range(NCORES):
        part = res.results[c]["o"].astype(np.float64)  # [128, RT, 8]
        maxA = part[:, :, 0:4].max(axis=2)  # [128, RT]
        maxB = part[:, :, 4:8].max(axis=2)
        idx = np.arange(RT)[None, :] * 128 + np.arange(128)[:, None]
        orig = (c * QPC + idx) % N1
        sqx = sq1[orig]
        rho_sq = sqx - 2.0 * maxA
        nu_sq = sqx - 2.0 * maxB
        rho_sq = np.maximum(rho_sq, 1e-20)
        nu_sq = np.maximum(nu_sq, 1e-20)
        total += 0.5 * (np.log(nu_sq) - np.log(rho_sq)).sum()
    base = np.log(N2 / (N1 - 1))
    return np.float32(base + (D / N1) * total)


# revision 12
# speedup vs baseline: 1.1676x; 1.0368x over previous
import sys

sys.path.insert(0, "/opt/trn_rl_repo")
import numpy as np

N1, N2, D = 8192, 8192, 256
NCORES = 8
QPC = N1 // NCORES  # queries per core (1024)
RT = QPC // 128  # row tiles per core (8)
GW = 2048  # group width (4 psum banks)
NEG = -1.0e30


def _build_nc():
    import concourse.bass as bass
    import concourse.tile as tile
    from concourse import mybir

    f32, f32r = mybir.dt.float32, mybir.dt.float32r
    nc = bass.Bass()
    dbx = nc.dram_tensor("dbx", [128, 2, 2 * N1], f32r, kind="ExternalInput")
    bf16 = mybir.dt.bfloat16
    nrmA = nc.dram_tensor("nrmA", [2, N1 + 128], bf16, kind="ExternalInput")
    nrmB = nc.dram_tensor("nrmB", [2, N1], bf16, kind="ExternalInput")
    dmask = nc.dram_tensor("dmask", [128, 4 * 512], f32, kind="ExternalInput")
    o = nc.dram_tensor("o", [128, RT, 8], f32, kind="ExternalOutput")

    with tile.TileContext(nc) as tc:
        with (
            tc.tile_pool(name="sb", bufs=1) as sb,
            tc.tile_pool(name="pp", bufs=1) as pp,
            tc.tile_pool(name="ps", bufs=2, space="PSUM") as ps,
        ):
            CW = 2 * GW  # chunk width (4096 cols)
            # small inputs first: the first norm-MM/mask-TT consumers must not
            # queue behind the 16MB database load on the same DMA engines
            tnrA = sb.tile([2, N1 + 128], bf16, tag="nrA")
            tnrB = sb.tile([2, N1], bf16, tag="nrB")
            tmk = sb.tile([128, 4 * 512], f32, tag="mk")
            nc.sync.dma_start(out=tnrA, in_=nrmA[:])
            nc.sync.dma_start(out=tnrB, in_=nrmB[:])
            nc.scalar.dma_start(out=tmk, in_=dmask[:])
            # first chunk split in half across both HWDGE engines so the
            # opening matmul group's data lands ASAP
            t000 = sb.tile([128, 2, GW], f32r, name="db000", tag="db000")
            nc.sync.dma_start(out=t000, in_=dbx[:, :, 0:GW])
            t00b = sb.tile([128, 2, GW], f32r, name="db00b", tag="db00b")
            nc.scalar.dma_start(out=t00b, in_=dbx[:, :, GW:CW])
            chunks = {(0, 0): (t000, t00b)}
            for side, h in [(0, 1), (1, 0), (1, 1)]:
                t = sb.tile([128, 2, CW], f32r, name=f"db{side}{h}", tag=f"db{side}{h}")
                off = side * N1 + h * CW
                eng = [None, nc.sync, nc.scalar, nc.sync][side * 2 + h]
                eng.dma_start(out=t, in_=dbx[:, :, off : off + CW])
                chunks[(side, h)] = t
            ones2 = tnrA[0:2, N1 : N1 + 128]
            # wait absorber: DVE observes the dmask DMA once, up front
            dum = sb.tile([128, 1], f32, tag="dum")
            nc.vector.tensor_copy(out=dum, in_=tmk[:, 0:1])
            parts = [pp.tile([128, 8], f32, name=f"part{m}", tag=f"part{m}") for m in range(RT)]
            tq = t000
            for side in range(2):
                for h in range(2):
                    tch = chunks[(side, h)]
                    for m in range(RT):
                        lhs = [tq[:, k, m * 128 : (m + 1) * 128] for k in (0, 1)]
                        part = parts[m]
                        for g2 in range(2):
                            g = h * 2 + g2
                            col = g * GW
                            if isinstance(tch, tuple):
                                tsrc, lcol = tch[g2], 0
                            else:
                                tsrc, lcol = tch, g2 * GW
                            pst = ps.tile([128, GW], f32, tag="pst")
                            for k in (0, 1):
                                for i in range(4):
                                    nc.tensor.matmul(
                                        out=pst[:, i * 512 : (i + 1) * 512],
                                        lhsT=lhs[k],
                                        rhs=tsrc[
                                            :, k, lcol + i * 512 : lcol + (i + 1) * 512
                                        ],
                                        start=(k == 0),
                                        stop=False,
                                    )
                            for i in range(4):
                                nc.tensor.matmul(
                                    out=pst[:, i * 512 : (i + 1) * 512],
                                    lhsT=ones2,
                                    rhs=(tnrA if side == 0 else tnrB)[
                                        0:2, col + i * 512 : col + (i + 1) * 512
                                    ],
                                    start=False,
                                    stop=True,
                                )
                            if side == 0 and g == 0:
                                i0, v = m // 4, m % 4
                                sl = pst[:, i0 * 512 : (i0 + 1) * 512]
                                nc.vector.tensor_add(
                                    out=sl, in0=sl, in1=tmk[:, v * 512 : (v + 1) * 512]
                                )
                            nc.vector.tensor_reduce(
                                out=part[:, side * 4 + g : side * 4 + g + 1],
                                in_=pst,
                                axis=mybir.AxisListType.X,
                                op=mybir.AluOpType.max,
                            )
            for m in range(RT):
                nc.sync.dma_start(out=o[:, m, :], in_=parts[m])

    from concourse.bass import _bass_rust

    _bass_rust.move_matmul_waits_to_ldweights(nc.m)
    _bass_rust.generate_event_semaphores(nc)
    return nc


def _bf16_hilo(x32):
    import ml_dtypes
    hi = x32.astype(ml_dtypes.bfloat16)
    lo = (x32 - hi.astype(np.float32)).astype(ml_dtypes.bfloat16)
    return hi, lo


def _prep_core(s1, s2T, sq2hi, sq2lo, c):
    s1p = np.roll(s1, -c * QPC, axis=0)
    dbx = np.empty((128, 2, 2 * N1), dtype=np.float32)
    s1pT = np.ascontiguousarray(s1p.T)
    for k in (0, 1):
        dbx[:, k, 0:N1] = s1pT[k * 128 : (k + 1) * 128]
        dbx[:, k, N1 : 2 * N1] = s2T[k * 128 : (k + 1) * 128]
    import ml_dtypes
    nA = (-0.5 * np.square(s1p.astype(np.float64)).sum(1)).astype(np.float32)
    hiA, loA = _bf16_hilo(nA)
    nrmA = np.ones((2, N1 + 128), dtype=ml_dtypes.bfloat16)
    nrmA[0, :N1], nrmA[1, :N1] = hiA, loA
    return dbx, nrmA


def kernel(s1, s2, k):
    assert int(k) == 1
    from concourse.bass_utils import run_bass_kernel_spmd

    s1 = np.asarray(s1, dtype=np.float32)
    s2 = np.asarray(s2, dtype=np.float32)
    nB = (-0.5 * np.square(s2.astype(np.float64)).sum(1)).astype(np.float32)
    hiB, loB = _bf16_hilo(nB)
    nrmB = np.stack([hiB, loB])
    s2T = np.ascontiguousarray(s2.T)
    dmask = np.zeros((128, 4 * 512), dtype=np.float32)
    for v in range(4):
        for p in range(128):
            dmask[p, v * 512 + v * 128 + p] = NEG

    nc = _build_nc()
    in_maps = []
    for c in range(NCORES):
        dbx, nrmA = _prep_core(s1, s2T, hiB, nB - hiB, c)
        in_maps.append({"dbx": dbx, "nrmA": nrmA, "nrmB": nrmB, "dmask": dmask})
    import os
    res = run_bass_kernel_spmd(
        nc, in_maps, core_ids=list(range(NCORES)),
        trace=os.environ.get("KBENCH_TRACE") == "1",
    )
    kernel.last_results = res

    # host epilogue (float64): rho/nu from per-group maxes, then the estimator
    sq1 = np.square(s1.astype(np.float64)).sum(1)
    total = 0.0
    for c in range(NCORES):
        part = res.results[c]["o"].astype(np.float64)  # [128, RT, 8]
        maxA = part[:, :, 0:4].max(axis=2)  # [128, RT]
        maxB = part[:, :, 4:8].max(axis=2)
        idx = np.arange(RT)[None, :] * 128 + np.arange(128)[:, None]
        orig = (c * QPC + idx) % N1
        sqx = sq1[orig]
        rho_sq = sqx - 2.0 * maxA
        nu_sq = sqx - 2.0 * maxB
        rho_sq = np.maximum(rho_sq, 1e-20)
        nu_sq = np.maximum(nu_sq, 1e-20)
        total += 0.5 * (np.log(nu_sq) - np.log(rho_sq)).sum()
    base = np.log(N2 / (N1 - 1))
    return np.float32(base + (D / N1) * total)


# revision 13
# speedup vs baseline: 1.1783x; 1.0092x over previous
import sys

sys.path.insert(0, "/opt/trn_rl_repo")
import numpy as np

N1, N2, D = 8192, 8192, 256
NCORES = 8
QPC = N1 // NCORES  # queries per core (1024)
RT = QPC // 128  # row tiles per core (8)
GW = 2048  # group width (4 psum banks)
NEG = -1.0e30


def _build_nc():
    import concourse.bass as bass
    import concourse.tile as tile
    from concourse import mybir

    f32, f32r = mybir.dt.float32, mybir.dt.float32r
    nc = bass.Bass()
    db000d = nc.dram_tensor("db000", [128, 2, GW], f32r, kind="ExternalInput")
    db00bd = nc.dram_tensor("db00b", [128, 2, GW], f32r, kind="ExternalInput")
    db01d = nc.dram_tensor("db01", [128, 2, 2 * GW], f32r, kind="ExternalInput")
    db10d = nc.dram_tensor("db10", [128, 2, 2 * GW], f32r, kind="ExternalInput")
    db11d = nc.dram_tensor("db11", [128, 2, 2 * GW], f32r, kind="ExternalInput")
    bf16 = mybir.dt.bfloat16
    nrmA = nc.dram_tensor("nrmA", [2, N1 + 128], bf16, kind="ExternalInput")
    nrmB = nc.dram_tensor("nrmB", [2, N1], bf16, kind="ExternalInput")
    dmask = nc.dram_tensor("dmask", [128, 4 * 512], f32, kind="ExternalInput")
    o = nc.dram_tensor("o", [128, RT, 8], f32, kind="ExternalOutput")

    with tile.TileContext(nc) as tc:
        with (
            tc.tile_pool(name="sb", bufs=1) as sb,
            tc.tile_pool(name="pp", bufs=1) as pp,
            tc.tile_pool(name="ps", bufs=2, space="PSUM") as ps,
        ):
            CW = 2 * GW  # chunk width (4096 cols)
            # small inputs first: the first norm-MM/mask-TT consumers must not
            # queue behind the 16MB database load on the same DMA engines
            tnrA = sb.tile([2, N1 + 128], bf16, tag="nrA")
            tnrB = sb.tile([2, N1], bf16, tag="nrB")
            tmk = sb.tile([128, 4 * 512], f32, tag="mk")
            nc.sync.dma_start(out=tnrA, in_=nrmA[:])
            nc.sync.dma_start(out=tnrB, in_=nrmB[:])
            nc.scalar.dma_start(out=tmk, in_=dmask[:])
            # first chunk split in half across both HWDGE engines so the
            # opening matmul group's data lands ASAP
            t000 = sb.tile([128, 2, GW], f32r, name="db000", tag="db000")
            nc.sync.dma_start(out=t000, in_=db000d[:])
            t00b = sb.tile([128, 2, GW], f32r, name="db00b", tag="db00b")
            nc.scalar.dma_start(out=t00b, in_=db00bd[:])
            chunks = {(0, 0): (t000, t00b)}
            for (side, h), dsrc, eng in [
                ((0, 1), db01d, nc.sync),
                ((1, 0), db10d, nc.scalar),
                ((1, 1), db11d, nc.sync),
            ]:
                t = sb.tile([128, 2, CW], f32r, name=f"db{side}{h}", tag=f"db{side}{h}")
                eng.dma_start(out=t, in_=dsrc[:])
                chunks[(side, h)] = t
            ones2 = tnrA[0:2, N1 : N1 + 128]
            # wait absorber: DVE observes the dmask DMA once, up front
            dum = sb.tile([128, 1], f32, tag="dum")
            nc.vector.tensor_copy(out=dum, in_=tmk[:, 0:1])
            parts = [pp.tile([128, 8], f32, name=f"part{m}", tag=f"part{m}") for m in range(RT)]
            tq = t000
            for side in range(2):
                for h in range(2):
                    tch = chunks[(side, h)]
                    for m in range(RT):
                        lhs = [tq[:, k, m * 128 : (m + 1) * 128] for k in (0, 1)]
                        part = parts[m]
                        for g2 in range(2):
                            g = h * 2 + g2
                            col = g * GW
                            if isinstance(tch, tuple):
                                tsrc, lcol = tch[g2], 0
                            else:
                                tsrc, lcol = tch, g2 * GW
                            pst = ps.tile([128, GW], f32, tag="pst")
                            for k in (0, 1):
                                for i in range(4):
                                    nc.tensor.matmul(
                                        out=pst[:, i * 512 : (i + 1) * 512],
                                        lhsT=lhs[k],
                                        rhs=tsrc[
                                            :, k, lcol + i * 512 : lcol + (i + 1) * 512
                                        ],
                                        start=(k == 0),
                                        stop=False,
                                    )
                            for i in range(4):
                                nc.tensor.matmul(
                                    out=pst[:, i * 512 : (i + 1) * 512],
                                    lhsT=ones2,
                                    rhs=(tnrA if side == 0 else tnrB)[
                                        0:2, col + i * 512 : col + (i + 1) * 512
                                    ],
                                    start=False,
                                    stop=True,
                                )
                            if side == 0 and g == 0:
                                i0, v = m // 4, m % 4
                                sl = pst[:, i0 * 512 : (i0 + 1) * 512]
                                nc.vector.tensor_add(
                                    out=sl, in0=sl, in1=tmk[:, v * 512 : (v + 1) * 512]
                                )
                            nc.vector.tensor_reduce(
                                out=part[:, side * 4 + g : side * 4 + g + 1],
                                in_=pst,
                                axis=mybir.AxisListType.X,
                                op=mybir.AluOpType.max,
                            )
            for m in range(RT):
                nc.sync.dma_start(out=o[:, m, :], in_=parts[m])

    from concourse.bass import _bass_rust

    _bass_rust.move_matmul_waits_to_ldweights(nc.m)
    _bass_rust.generate_event_semaphores(nc)
    return nc


def _bf16_hilo(x32):
    import ml_dtypes
    hi = x32.astype(ml_dtypes.bfloat16)
    lo = (x32 - hi.astype(np.float32)).astype(ml_dtypes.bfloat16)
    return hi, lo


def _prep_core(s1, s2T, sq2hi, sq2lo, c):
    s1p = np.roll(s1, -c * QPC, axis=0)
    dbx = np.empty((128, 2, 2 * N1), dtype=np.float32)
    s1pT = np.ascontiguousarray(s1p.T)
    for k in (0, 1):
        dbx[:, k, 0:N1] = s1pT[k * 128 : (k + 1) * 128]
        dbx[:, k, N1 : 2 * N1] = s2T[k * 128 : (k + 1) * 128]
    CW = 2 * GW
    dbd = {
        "db000": np.ascontiguousarray(dbx[:, :, 0:GW]),
        "db00b": np.ascontiguousarray(dbx[:, :, GW:CW]),
        "db01": np.ascontiguousarray(dbx[:, :, CW : 2 * CW]),
        "db10": np.ascontiguousarray(dbx[:, :, N1 : N1 + CW]),
        "db11": np.ascontiguousarray(dbx[:, :, N1 + CW : N1 + 2 * CW]),
    }
    dbx = dbd
    import ml_dtypes
    nA = (-0.5 * np.square(s1p.astype(np.float64)).sum(1)).astype(np.float32)
    hiA, loA = _bf16_hilo(nA)
    nrmA = np.ones((2, N1 + 128), dtype=ml_dtypes.bfloat16)
    nrmA[0, :N1], nrmA[1, :N1] = hiA, loA
    return dbx, nrmA


def kernel(s1, s2, k):
    assert int(k) == 1
    from concourse.bass_utils import run_bass_kernel_spmd

    s1 = np.asarray(s1, dtype=np.float32)
    s2 = np.asarray(s2, dtype=np.float32)
    nB = (-0.5 * np.square(s2.astype(np.float64)).sum(1)).astype(np.float32)
    hiB, loB = _bf16_hilo(nB)
    nrmB = np.stack([hiB, loB])
    s2T = np.ascontiguousarray(s2.T)
    dmask = np.zeros((128, 4 * 512), dtype=np.float32)
    for v in range(4):
        for p in range(128):
            dmask[p, v * 512 + v * 128 + p] = NEG

    nc = _build_nc()
    in_maps = []
    for c in range(NCORES):
        dbd, nrmA = _prep_core(s1, s2T, hiB, nB - hiB, c)
        in_maps.append({**dbd, "nrmA": nrmA, "nrmB": nrmB, "dmask": dmask})
    import os
    res = run_bass_kernel_spmd(
        nc, in_maps, core_ids=list(range(NCORES)),
        trace=os.environ.get("KBENCH_TRACE") == "1",
    )
    kernel.last_results = res

    # host epilogue (float64): rho/nu from per-group maxes, then the estimator
    sq1 = np.square(s1.astype(np.float64)).sum(1)
    total = 0.0
    for c in range(NCORES):
        part = res.results[c]["o"].astype(np.float64)  # [128, RT, 8]
        maxA = part[:, :, 0:4].max(axis=2)  # [128, RT]
        maxB = part[:, :, 4:8].max(axis=2)
        idx = np.arange(RT)[None, :] * 128 + np.arange(128)[:, None]
        orig = (c * QPC + idx) % N1
        sqx = sq1[orig]
        rho_sq = sqx - 2.0 * maxA
        nu_sq = sqx - 2.0 * maxB
        rho_sq = np.maximum(rho_sq, 1e-20)
        nu_sq = np.maximum(nu_sq, 1e-20)
        total += 0.5 * (np.log(nu_sq) - np.log(rho_sq)).sum()
    base = np.log(N2 / (N1 - 1))
    return np.float32(base + (D / N1) * total)
